# revision 2
# baseline (speedup 1.0000x reference)
"""ContextualAttention TRN2 kernel v3.

8 cores = 2 batches x 4 strips of 16 grid rows. Per core:
  phase A: E = exp(10*(wn8^T @ pr8 - d_q) + mask) via fp8 DoubleRow GEMM
           (stabilizer -d_q and mask penalty folded into padded
           contraction rows), q in width-65 padded rows, 18 rows/core.
  phase B: colsum via ones^T @ acc matmul, r = 1/(colsum+eps),
           R = broadcast(r), A = E*R.
  phase C: Ahat[p,q0] = sum_{(di,dj)} A[p-d, q0-d] built with 0/1
           shift-matrix matmuls (holes enforce grid-row boundaries),
           then 4 parity GEMMs out[par][c,q0] = xs_par^T @ Ahat give the
           final image quadrants directly (deconv overlap-add folded in,
           /4 folded into xs).  p'=64 boundary row/col/corner handled by
           a gathered correction tile.
"""
import numpy as np
import ml_dtypes

import concourse.bass as bass
import concourse.bacc as bacc
import concourse.mybir as mybir
from concourse import tile
from concourse.bass_utils import run_bass_kernel_spmd

F32 = mybir.dt.float32
BF16 = mybir.dt.bfloat16
F8 = mybir.dt.float8e4
DRMODE = mybir.MatmulPerfMode.DoubleRow
AFT = mybir.ActivationFunctionType

B, C, H, W = 2, 128, 128, 128
RATE, BS = 2, 3
Hr = Wr = 64
L = Hr * Wr                    # 4096
F = C * BS * BS                # 1152
KPAD = 1280                    # 5 DoubleRow k-tiles of 256
NKT = 5
NPT = 32
NROWS, WP = 18, 65
QC = NROWS * WP                # 1170
QA = QC + 1                    # 1171 (leading zero col)
QH = 17 * WP                   # 1105
SCALE = 10.0
EPS = 1e-4
BIG_D = 40.0
N_CORES = 8
CH_A = [(0, 512), (512, 512), (1024, QC - 1024)]
CH_H = [(0, 512), (512, 512), (1024, QH - 1024)]
# shift-matrix indices
SH_I, SH_P1, SH_M64, SH_M65, SH_C64, SH_C65, SH_B0, SH_B1, SH_R0, SH_R1 = range(10)

_CACHE = {}


def _mov_ap(t, row0, par_i0, par_j0, h):
    """Moving AP: 8 rows x 64 cols of t's width-65 row grid.

    row0: extra row offset (0 for QH-indexed tiles)."""
    base = ((1 - par_i0) + 8 * h + row0) * WP + (1 - par_j0)
    return (t[:, base:base + 8 * WP]
            .rearrange("p (r w) -> p r w", w=WP)[:, :, 0:64])


def _build_nc():
    nc = bacc.Bacc(None)
    wn8_d = nc.declare_dram_parameter("wn8", [128, NPT * NKT * 2 * 128], F8,
                                      isOutput=False)
    prq8_d = nc.declare_dram_parameter("prq8", [128, NKT * 2 * QC], F8,
                                       isOutput=False)
    xs_d = nc.declare_dram_parameter("xs", [L, 512], BF16, isOutput=False)
    xsb_d = nc.declare_dram_parameter("xsb", [128, 512], BF16, isOutput=False)
    xsc_d = nc.declare_dram_parameter("xsc", [1, 512], BF16, isOutput=False)
    sh_d = nc.declare_dram_parameter("shifts", [128, 10 * 128], BF16,
                                     isOutput=False)
    col_d = nc.declare_dram_parameter("col", [512, 1024], F32, isOutput=True)

    with tile.TileContext(nc) as tc:
        with (
            tc.tile_pool(name="epool", bufs=NPT) as epool,
            tc.tile_pool(name="hpool", bufs=NPT) as hpool,
            tc.tile_pool(name="aring", bufs=4) as apool,
            tc.tile_pool(name="a2ring", bufs=3) as a2pool,
            tc.tile_pool(name="lhs", bufs=3) as lhspool,
            tc.tile_pool(name="const", bufs=1) as cpool,
            tc.tile_pool(name="outs", bufs=2) as opool,
            tc.tile_pool(name="ps", bufs=8, space="PSUM") as pspool,
        ):
            prq_sb = cpool.tile([128, NKT * 2 * QC], F8)
            sh_sb = cpool.tile([128, 10 * 128], BF16)
            xsb_sb = cpool.tile([128, 512], BF16)
            xsc_sb = cpool.tile([1, 512], BF16)
            ones_col = cpool.tile([128, 1], F32)
            ones1 = cpool.tile([1, 128], BF16)
            acc = cpool.tile([128, QC], F32)
            rrow = cpool.tile([1, QC], BF16)
            R_sb = cpool.tile([128, QC], F32)
            G_sb = cpool.tile([128, QA], BF16)
            corner = cpool.tile([1, QA], BF16)
            hb = cpool.tile([128, QH + 1], BF16)

            # first-needed loads lead the queues
            lhs_tiles = []
            lhs = lhspool.tile([128, NKT * 2 * 128], F8, tag="lhs")
            nc.gpsimd.dma_start(lhs[:], wn8_d[:, 0:1280])
            lhs_tiles.append(lhs)
            for k in range(NKT):
                nc.sync.dma_start(prq_sb[:, k * 2 * QC:(k + 1) * 2 * QC],
                                  prq8_d[:, k * 2 * QC:(k + 1) * 2 * QC])
            nc.gpsimd.dma_start(sh_sb[:], sh_d[:])
            nc.gpsimd.dma_start(xsb_sb[:], xsb_d[:])
            nc.gpsimd.dma_start(xsc_sb[:], xsc_d[:])
            nc.gpsimd.memset(ones_col[:], 1.0)
            nc.gpsimd.memset(ones1[:], 1.0)
            nc.gpsimd.memset(acc[:], 0.0)
            nc.gpsimd.memset(G_sb[64:128, :], 0.0)

            # ---------------- phase A ----------------
            _sidA = nc.enter_named_scope("phaseA", False)[0]
            e_tiles = []
            for pt in range(NPT):
                if pt + 1 < NPT:
                    lhs = lhspool.tile([128, NKT * 2 * 128], F8, tag="lhs")
                    (nc.sync if pt % 2 else nc.gpsimd).dma_start(
                        lhs[:], wn8_d[:, (pt + 1) * 1280:(pt + 2) * 1280])
                    lhs_tiles.append(lhs)
                lhs = lhs_tiles[pt]
                et = epool.tile([128, QA], BF16, tag="e", name=f"e{pt}")
                nc.gpsimd.memset(et[:, 0:1], 0.0)
                for (off, cl) in CH_A:
                    ps = pspool.tile([128, 512], F32, tag="ps")
                    for k in range(NKT):
                        nc.tensor.matmul(
                            ps[:, :cl],
                            lhs[:, k * 256:(k + 1) * 256]
                            .rearrange("p (i m) -> p i m", i=2),
                            prq_sb[:, k * 2 * QC:(k + 1) * 2 * QC]
                            .rearrange("p (i q) -> p i q", i=2)[:, :, off:off + cl],
                            start=(k == 0), stop=(k == NKT - 1),
                            perf_mode=DRMODE)
                    nc.scalar.activation(et[:, 1 + off:1 + off + cl],
                                         ps[:, :cl], AFT.Exp, scale=SCALE)
                if pt < NPT - 1:
                    nc.vector.tensor_add(acc[:], acc[:], et[:, 1:])
                e_tiles.append(et)

            nc.leave_named_scope("phaseA", _sidA, False)
            _sidB = nc.enter_named_scope("phaseB", False)[0]
            # ---------------- phase B (chunk-pipelined) ----------------
            et = e_tiles[NPT - 1]
            for ci, (off, cl) in enumerate(CH_A):
                nc.vector.tensor_add(acc[:, off:off + cl], acc[:, off:off + cl],
                                     et[:, 1 + off:1 + off + cl])
                csp = pspool.tile([128, 512], F32, tag="ps", name=f"cs{ci}")
                nc.tensor.matmul(csp[0:1, :cl], ones_col[:], acc[:, off:off + cl],
                                 start=True, stop=True)
                with nc.allow_low_precision(reason="r in bf16 is 0.4% scale"):
                    nc.vector.tensor_scalar_add(rrow[:, off:off + cl],
                                                csp[0:1, :cl], 1e-30)
            with nc.allow_low_precision(reason="r in bf16 is 0.4% scale"):
                nc.vector.reciprocal(rrow[:], rrow[:])
            for ci, (off, cl) in enumerate(CH_A):
                rp = pspool.tile([128, 512], F32, tag="ps", name=f"rb{ci}")
                nc.tensor.matmul(rp[:, :cl], ones1[:], rrow[0:1, off:off + cl],
                                 start=True, stop=True)
                nc.vector.tensor_copy(R_sb[:, off:off + cl], rp[:, :cl])

            nc.leave_named_scope("phaseB", _sidB, False)
            _sidH = nc.enter_named_scope("ahat", False)[0]
            # ---------------- Ahat build ----------------
            a_tiles = {}
            a2_tiles = {}
            xs_tiles = []

            def scale_a(pt):
                at = apool.tile([128, QA], BF16, tag="a", name=f"a{pt}")
                nc.gpsimd.memset(at[:, 0:1], 0.0)
                nc.vector.tensor_mul(at[:, 1:], e_tiles[pt][:, 1:], R_sb[:])
                nc.scalar.dma_start(G_sb[2 * pt:2 * pt + 1, :], at[63:64, :])
                nc.scalar.dma_start(G_sb[2 * pt + 1:2 * pt + 2, :],
                                    at[127:128, :])
                a_tiles[pt] = at
                xt = epool.tile([128, QA], BF16, tag="e", name=f"xs{pt}")
                (nc.gpsimd if pt % 2 else nc.sync).dma_start(
                    xt[:, 0:512], xs_d[pt * 128:(pt + 1) * 128, :])
                xs_tiles.append(xt)

            def make_a2(pt):
                # A2_pt = grid rows {2pt+1, 2pt+2} = A1_pt[64:], A1_{pt+1}[:64]
                a2 = a2pool.tile([128, QA], BF16, tag="a2", name=f"a2_{pt}")
                nc.sync.dma_start(a2[0:64, :], a_tiles[pt][64:128, :])
                nc.gpsimd.dma_start(a2[64:128, :], a_tiles[pt + 1][0:64, :])
                a2_tiles[pt] = a2

            scale_a(0)
            scale_a(1)
            make_a2(0)
            h_tiles = []
            for pt in range(NPT):
                pieces = [(SH_I, a_tiles[pt], 66), (SH_P1, a_tiles[pt], 65)]
                if pt > 0:
                    pieces += [(SH_I, a2_tiles[pt - 1], 1),
                               (SH_P1, a2_tiles[pt - 1], 0)]
                else:
                    pieces += [(SH_M64, a_tiles[0], 1),
                               (SH_M65, a_tiles[0], 0)]
                pss = [pspool.tile([128, 512], F32, tag="ps",
                                   name=f"h{pt}_{i}") for i in range(3)]
                for si, (s, src, cb) in enumerate(pieces):
                    for ci, (off, cl) in enumerate(CH_H):
                        nc.tensor.matmul(
                            pss[ci][:, :cl], sh_sb[:, s * 128:(s + 1) * 128],
                            src[:, cb + off:cb + off + cl],
                            start=(si == 0), stop=(si == len(pieces) - 1))
                ht = hpool.tile([128, QH + 1], BF16, tag="h", name=f"h{pt}")
                for ci, (off, cl) in enumerate(CH_H):
                    if ci == 1:
                        nc.vector.tensor_copy(ht[:, off:off + cl],
                                              pss[ci][:, :cl])
                    else:
                        nc.scalar.activation(ht[:, off:off + cl],
                                             pss[ci][:, :cl], AFT.Copy)
                h_tiles.append(ht)

                if pt + 2 < NPT:
                    scale_a(pt + 2)
                    make_a2(pt + 1)

            # ---------------- boundary tile ----------------
            a31 = a_tiles[NPT - 1]
            nc.scalar.dma_start(corner[0:1, :], a31[127:128, :])
            piecesB = [(SH_B0, a31, 1), (SH_B1, a31, 0),
                       (SH_R0, G_sb, 65), (SH_R1, G_sb, 0)]
            psb = [pspool.tile([128, 512], F32, tag="ps", name=f"hb{i}")
                   for i in range(3)]
            for si, (s, src, cb) in enumerate(piecesB):
                for ci, (off, cl) in enumerate(CH_H):
                    nc.tensor.matmul(
                        psb[ci][:, :cl], sh_sb[:, s * 128:(s + 1) * 128],
                        src[:, cb + off:cb + off + cl],
                        start=(si == 0), stop=(si == 3))
            for ci, (off, cl) in enumerate(CH_H):
                nc.scalar.activation(hb[:, off:off + cl], psb[ci][:, :cl],
                                     AFT.Copy)

            nc.leave_named_scope("ahat", _sidH, False)
            _sidP = nc.enter_named_scope("parity", False)[0]
            # ---------------- parity GEMMs (par-outer, early epilogues) ----
            for par in range(4):
                i0, j0 = par // 2, par % 2
                psps = [pspool.tile([128, 512], F32, tag="ps",
                                    name=f"pp{par}{h}") for h in range(2)]
                for pt in range(NPT):
                    for h in range(2):
                        nc.tensor.matmul(
                            psps[h][:], xs_tiles[pt][:, par * 128:(par + 1) * 128],
                            _mov_ap(h_tiles[pt], 0, i0, j0, h),
                            start=(pt == 0), stop=False)
                for h in range(2):
                    nc.tensor.matmul(
                        psps[h][:], xsb_sb[:, par * 128:(par + 1) * 128],
                        _mov_ap(hb, 0, i0, j0, h), start=False, stop=False)
                    nc.tensor.matmul(
                        psps[h][:], xsc_sb[0:1, par * 128:(par + 1) * 128],
                        _mov_ap(corner, 0, i0, j0, h), start=False, stop=True)
                    ot = opool.tile([128, 512], F32, tag="o")
                    nc.vector.tensor_copy(ot[:], psps[h][:])
                    nc.scalar.dma_start(
                        col_d[par * 128:(par + 1) * 128,
                              h * 512:(h + 1) * 512], ot[:])
            nc.leave_named_scope("parity", _sidP, False)
    nc.compile()
    return nc


# ---------------------------------------------------------------- host side
def _f8(x):
    return np.asarray(x, np.float32).astype(ml_dtypes.float8_e4m3fn)


def _bf(x):
    return np.asarray(x, np.float32).astype(ml_dtypes.bfloat16)


def _shift_host():
    def mk(pairs, rows=128):
        m = np.zeros((rows, 128), np.float32)
        for k, mm in pairs:
            m[k, mm] = 1.0
        if rows < 128:
            m = np.vstack([m, np.zeros((128 - rows, 128), np.float32)])
        return m
    mats = [None] * 10
    mats[SH_I] = np.eye(128, dtype=np.float32)
    mats[SH_P1] = mk([(k, k + 1) for k in range(127) if (k + 1) % 64 != 0])
    mats[SH_M64] = mk([(k, k + 64) for k in range(64)])
    mats[SH_M65] = mk([(k, k + 65) for k in range(63)])
    mats[SH_C64] = mk([(k, k - 64) for k in range(64, 128)])
    mats[SH_C65] = mk([(k, k - 63) for k in range(64, 127)
                       if (k - 63) % 64 != 0])
    mats[SH_B0] = mk([(k, k - 64) for k in range(64, 128)])
    mats[SH_B1] = mk([(k, k - 63) for k in range(64, 127)])
    mats[SH_R0] = mk([(k, 64 + k) for k in range(64)], rows=64)
    mats[SH_R1] = mk([(k, 65 + k) for k in range(63)], rows=64)
    arr = np.stack(mats)                       # [10, 128, 128] (k, m)
    return _bf(arr.transpose(1, 0, 2).reshape(128, 10 * 128))


def _pad_cols(arr_grid, g):
    lead = arr_grid.shape[:-2]
    out = np.zeros(lead + (NROWS, WP), np.float32)
    r0 = 16 * g - 1
    lo, hi = max(0, r0), min(64, r0 + NROWS)
    out[..., lo - r0:hi - r0, :64] = arr_grid[..., lo:hi, :]
    return out


def _host_prep(x, mask):
    """Returns per-batch dict of shared operands + per-core prq8."""
    batches = []
    for b in range(B):
        xr = x[b, :, ::RATE, ::RATE]
        xrp = np.pad(xr, ((0, 0), (1, 1), (1, 1)))
        pr = np.empty((9, C, Hr, Wr), np.float32)
        for di in range(3):
            for dj in range(3):
                pr[di * 3 + dj] = xrp[:, di:di + Hr, dj:dj + Wr]
        pr = pr.reshape(F, Hr, Wr)
        denom = np.sqrt((pr.reshape(F, L) ** 2).sum(0, dtype=np.float64)
                        .astype(np.float32)
                        + np.float32(F * EPS)).reshape(Hr, Wr)

        mr = mask[b, :, ::RATE, ::RATE]
        mrp = np.pad(mr, ((0, 0), (1, 1), (1, 1)))
        msum = np.zeros((Hr, Wr), np.float32)
        for di in range(3):
            for dj in range(3):
                msum += mrp[0, di:di + Hr, dj:dj + Wr]
        mfilt = (msum == 0.0).astype(np.float32)

        wn = (pr.reshape(F, L) / denom.reshape(1, L)) * mfilt.reshape(1, L)
        wn8 = np.zeros((KPAD, L), np.float32)
        wn8[:F] = _f8(wn).astype(np.float32)
        wn8[F] = 1.0
        wn8[F + 1] = 96.0 * (1.0 - mfilt.reshape(L))
        wn8_t = _f8(wn8.reshape(NKT, 2, 128, NPT, 128)
                    .transpose(2, 3, 0, 1, 4).reshape(128, NPT * 1280))

        d8 = _f8(denom).astype(np.float32)

        xp = np.pad(x[b], ((0, 0), (1, 1), (1, 1)))
        xs65 = np.zeros((65, 65, 4, C), np.float32)
        for i0 in range(2):
            for j0 in range(2):
                sl = xp[:, i0:i0 + 130:2, j0:j0 + 130:2]
                xs65[:, :, i0 * 2 + j0, :] = sl.transpose(1, 2, 0)
        xs65 *= 0.25

        cores = []
        for g in range(4):
            prs = _pad_cols(pr, g).reshape(F, QC)
            ds = _pad_cols(d8, g).reshape(QC)
            vmask = _pad_cols(np.ones((Hr, Wr), np.float32), g).reshape(QC)
            stab = np.where(vmask > 0, ds, BIG_D)
            prq8 = np.zeros((KPAD, QC), np.float32)
            prq8[:F] = _f8(prs).astype(np.float32)
            prq8[F] = -stab
            prq8[F + 1] = -96.0
            cores.append(_f8(prq8.reshape(NKT, 2, 128, QC)
                             .transpose(2, 0, 1, 3).reshape(128, NKT * 2 * QC)))

        batches.append({
            "wn8": wn8_t,
            "prq8_by_g": cores,
            "xs": _bf(xs65[:64, :64].reshape(L, 4 * C)),
            "xsb": _bf(np.concatenate([xs65[64, :64], xs65[:64, 64]])
                       .reshape(128, 4 * C)),
            "xsc": _bf(xs65[64, 64].reshape(1, 4 * C)),
        })
    return batches


def kernel(x, mask):
    x = np.asarray(x, np.float32)
    mask = np.asarray(mask, np.float32)
    if "nc" not in _CACHE:
        _CACHE["nc"] = _build_nc()
    nc = _CACHE["nc"]

    shifts = _shift_host()
    batches = _host_prep(x, mask)
    in_maps = []
    for core in range(N_CORES):
        b, g = divmod(core, 4)
        bb = batches[b]
        in_maps.append({
            "wn8": bb["wn8"],
            "prq8": bb["prq8_by_g"][g],
            "xs": bb["xs"],
            "xsb": bb["xsb"],
            "xsc": bb["xsc"],
            "shifts": shifts,
        })
    _CACHE["in_maps"] = in_maps
    res = run_bass_kernel_spmd(nc, in_maps, list(range(N_CORES)))

    out = np.empty((B, C, H, W), np.float32)
    for core in range(N_CORES):
        b, g = divmod(core, 4)
        col = res.results[core]["col"].reshape(4, C, 16, 64)
        for i0 in range(2):
            for j0 in range(2):
                ys = 32 * g + (1 - i0)
                out[b, :, ys:ys + 32:2, (1 - j0)::2] = col[i0 * 2 + j0]
    return out
